# revision 1
# baseline (speedup 1.0000x reference)
"""Trainium2 Bass kernel for nn_AdvancedHybridGNN (hybrid GCN+GAT, N=30000, E=600000).

Strategy (8 NeuronCores, graph/data parallel):
- Nodes padded to 30720 = 8 cores x 30 blocks x 128. Edges (+self loops) sorted by
  destination, grouped per dst-block, padded per block to T_blk tiles of 128 edges.
- Per aggregation layer: each core builds the gather table for its node shard
  (dense matmul in feature-major layout), cores AllGather the table to HBM, then
  each core gathers source rows per edge tile (dma_gather, 512B/768B rows) and
  reduces them per dst block with a one-hot selection matrix on the TensorEngine
  (S^T @ messages accumulated in PSUM).
- GCN: sym-norm folded as dis[src] into the table and dis[dst] into the epilogue.
- GAT: table rows [xh | asrc | adst]; adst fetched by a second (dst-indexed)
  gather; segment softmax without max-subtraction (exact up to fp), divide by
  segment sum after aggregation. exp(leaky(z)) == max(exp(z), exp(0.2 z)).
- All BatchNorms (eval mode) folded into per-feature affines; heads chained in
  feature-major layout so bias/activation are per-partition ops.
"""
import numpy as np
import sys

sys.path.insert(0, "/opt/trn_rl_repo")

import concourse.bacc as bacc
import concourse.bass as bass
import concourse.mybir as mybir
import concourse.tile as tile
from concourse import library_config
from concourse.masks import make_identity
from concourse.bass_utils import run_bass_kernel_spmd

F32 = mybir.dt.float32
I16 = mybir.dt.int16
AF = mybir.ActivationFunctionType
ALU = mybir.AluOpType

N = 30000
NPAD = 30720
E = 600000
IN = 64
HID = 128
EPS = 1e-5
HEADS = (4, 4, 1)
NC = 8
BLK = 128
NBLK = NPAD // BLK      # 240
BPC = NBLK // NC        # 30 blocks per core
GATW = 192              # gat table row width (multiple of 64 for 256B stride)


# ----------------------------------------------------------------------------
# Host-side graph preprocessing
# ----------------------------------------------------------------------------

def preprocess_graph(edge_index):
    src = np.concatenate([edge_index[0].astype(np.int64),
                          np.arange(N, dtype=np.int64)])
    dst = np.concatenate([edge_index[1].astype(np.int64),
                          np.arange(N, dtype=np.int64)])
    deg = np.bincount(dst, minlength=NPAD)
    dis = np.zeros(NPAD, np.float32)
    m = deg > 0
    dis[m] = (1.0 / np.sqrt(deg[m].astype(np.float64))).astype(np.float32)

    order = np.argsort(dst, kind="stable")
    src_s = src[order]
    dst_s = dst[order]
    blk_of = dst_s // BLK
    counts = np.bincount(blk_of, minlength=NBLK)
    T_blk = max(1, int(np.ceil(counts.max() / BLK)))
    EPB = T_blk * BLK
    src_pad = np.zeros((NBLK, EPB), np.int64)
    # pad edges point at their own block base: valid row everywhere (also
    # after the shard-local shift for adst gathers); S kills their output
    dst_pad = np.tile((np.arange(NBLK, dtype=np.int64) * BLK)[:, None], (1, EPB))
    dstloc = np.full((NBLK, EPB), 255.0, np.float32)
    starts = np.zeros(NBLK + 1, np.int64)
    np.cumsum(counts, out=starts[1:])
    for b in range(NBLK):
        c = counts[b]
        s0 = starts[b]
        src_pad[b, :c] = src_s[s0:s0 + c]
        dst_pad[b, :c] = dst_s[s0:s0 + c]
        dstloc[b, :c] = (dst_s[s0:s0 + c] - b * BLK).astype(np.float32)
    return src_pad, dst_pad, dstloc, dis, T_blk


def wrap_idx_core(idx_tiles):
    """idx_tiles: [T, 128] int -> int16 gather layout [128, T*8].

    Within each tile, position i -> partition i%16, free col tile*8 + i//16,
    replicated across the 8 partition groups.
    """
    T = idx_tiles.shape[0]
    a = idx_tiles.reshape(T, 8, 16).transpose(2, 0, 1).reshape(16, T * 8)
    return np.tile(a, (8, 1)).astype(np.int16)


def bn_fold(g):
    return g / np.sqrt(1.0 + EPS)


def make_blockdiag(a, heads):
    C = HID // heads
    bd = np.zeros((HID, heads), np.float32)
    for h in range(heads):
        bd[h * C:(h + 1) * C, h] = a[h * C:(h + 1) * C]
    return bd


# ----------------------------------------------------------------------------
# Device program
# ----------------------------------------------------------------------------

NQ = 4  # SWDGE queues for gather parallelism
CHSZ = 8   # tiles per gather chunk


def chunks_of(T_blk):
    out = [CHSZ] * (T_blk // CHSZ)
    if T_blk % CHSZ:
        out.append(T_blk % CHSZ)
    return out


def build_program(T_blk, ngcn=3, ngat=3, do_heads=True):
    """Build the SPMD Bass program (same for all 8 cores)."""
    nc = bacc.Bacc("TRN2", num_swdge_queues=NQ)
    TILES = BPC * T_blk
    # chunking of each block's tiles: small chunks so >=4 gathers are in
    # flight across the NQ SWDGE queues
    chunks = chunks_of(T_blk)
    CHMAX = max(chunks)

    # ---- inputs ----
    x_fm = nc.dram_tensor("x_fm", [IN, BPC * BLK], F32, kind="ExternalInput")
    idx_src = nc.dram_tensor("idx_src", [128, TILES * 8], I16, kind="ExternalInput")
    idx_dst = nc.dram_tensor("idx_dst", [128, TILES * 8], I16, kind="ExternalInput")
    dstloc = nc.dram_tensor("dstloc", [128, TILES], F32, kind="ExternalInput")
    iota_rep = nc.dram_tensor("iota_rep", [128, CHMAX * 128], F32, kind="ExternalInput")
    dis_in = nc.dram_tensor("dis", [128, BPC], F32, kind="ExternalInput")

    gcn_in_w = nc.dram_tensor("gcn_in_w", [IN, HID], F32, kind="ExternalInput")
    gcn_res_w = nc.dram_tensor("gcn_res_w", [IN, HID], F32, kind="ExternalInput")
    gat_in_w = nc.dram_tensor("gat_in_w", [IN, HID], F32, kind="ExternalInput")
    gcn_w = nc.dram_tensor("gcn_w", [3, HID, HID], F32, kind="ExternalInput")
    gat_wg = [nc.dram_tensor(f"gat_wg{i}", [HID, HID + 2 * HEADS[i]], F32,
                             kind="ExternalInput") for i in range(3)]
    # per-partition [128,1]-style vectors, packed as [128, K]
    vecs = nc.dram_tensor("vecs", [128, 32], F32, kind="ExternalInput")
    # head weights
    gcn_c1w = nc.dram_tensor("gcn_c1w", [HID, 64], F32, kind="ExternalInput")
    gcn_c2w = nc.dram_tensor("gcn_c2w", [64, 32], F32, kind="ExternalInput")
    gcn_c3w = nc.dram_tensor("gcn_c3w", [32, HID], F32, kind="ExternalInput")
    gat_c1w = nc.dram_tensor("gat_c1w", [HID, 64], F32, kind="ExternalInput")
    gat_c2w = nc.dram_tensor("gat_c2w", [64, 32], F32, kind="ExternalInput")
    gat_c3w = nc.dram_tensor("gat_c3w", [32, HID], F32, kind="ExternalInput")
    fin_w = nc.dram_tensor("fin_w", [HID, 64], F32, kind="ExternalInput")
    fin2_w = nc.dram_tensor("fin2_w", [64, 2], F32, kind="ExternalInput")

    out_fm = nc.dram_tensor("out_fm", [2, BPC * BLK], F32, kind="ExternalOutput")

    # vecs column map (host must match):
    # 0: gcn_in_b, 1: gcn_res_b, 2: gat_in_b
    # 3,4,5:  gcn bn scale g' per layer
    # 6,7,8:  gcn bn bias  (b*g'+b') per layer
    # 9,10,11: gat bn scale per layer
    # 12,13,14: gat bn bias per layer
    # 15: gcn_c1b (64), 16: gcn_c2b (32), 17: fused bias (0.6c3b+0.4c3b) (128)
    # 18: gat_c1b (64), 19: gat_c2bf (32)
    # 20: fin_b (64), 21: fin2_bf (2)
    VC = dict(gcn_in_b=0, gcn_res_b=1, gat_in_b=2,
              gcn_g=[3, 4, 5], gcn_bc=[6, 7, 8],
              gat_g=[9, 10, 11], gat_bc=[12, 13, 14],
              gcn_c1b=15, gcn_c2b=16, fused_b=17,
              gat_c1b=18, gat_c2b=19, fin_b=20, fin2_b=21)

    with tile.TileContext(nc) as tc:
        with (
            tc.tile_pool(name="const", bufs=1) as cpool,
            tc.tile_pool(name="state", bufs=1) as spool,
            tc.tile_pool(name="work", bufs=3) as wpool,
            tc.tile_pool(name="gath", bufs=6) as gpool,
            tc.tile_pool(name="stage", bufs=2) as stpool,
            tc.tile_pool(name="psA", bufs=2, space="PSUM") as psA,      # agg accs
            tc.tile_pool(name="psB", bufs=1, space="PSUM") as psB,      # table mm
            tc.tile_pool(name="psF", bufs=1, space="PSUM") as psF,      # fusion acc
            tc.tile_pool(name="psC", bufs=2, space="PSUM") as psC,      # transients
            tc.tile_pool(name="dram", bufs=1, space="DRAM") as dram,
        ):
            nc.gpsimd.load_library(library_config.mlp)

            # ---- constants into SBUF ----
            _ldn = [0]

            def ld(shape, dt, src):
                _ldn[0] += 1
                t = cpool.tile(shape, dt, tag=f"c{_ldn[0]}")
                nc.sync.dma_start(t[:], src)
                return t

            idxS = ld([128, TILES * 8], I16, idx_src[:])
            idxD = ld([128, TILES * 8], I16, idx_dst[:])
            dl = ld([128, TILES], F32, dstloc[:])
            iota = ld([128, CHMAX * 128], F32, iota_rep[:])
            dis = ld([128, BPC], F32, dis_in[:])
            vec = ld([128, 32], F32, vecs[:])
            w_in = ld([IN, HID], F32, gcn_in_w[:])
            w_res = ld([IN, HID], F32, gcn_res_w[:])
            w_gin = ld([IN, HID], F32, gat_in_w[:])
            wg = [ld([HID, HID], F32, gcn_w[i, :, :]) for i in range(3)]
            wa = [ld([HID, HID + 2 * HEADS[i]], F32, gat_wg[i][:]) for i in range(3)]
            hw = {}
            for nm, hnd, shp in (
                ("gcn_c1w", gcn_c1w, [HID, 64]), ("gcn_c2w", gcn_c2w, [64, 32]),
                ("gcn_c3w", gcn_c3w, [32, HID]), ("gat_c1w", gat_c1w, [HID, 64]),
                ("gat_c2w", gat_c2w, [64, 32]), ("gat_c3w", gat_c3w, [32, HID]),
                ("fin_w", fin_w, [HID, 64]), ("fin2_w", fin2_w, [64, 2]),
            ):
                hw[nm] = ld(shp, F32, hnd[:])
            ident = cpool.tile([128, 128], F32, tag="ident")
            make_identity(nc, ident[:])
            x_sb = ld([IN, BPC * BLK], F32, x_fm[:])

            def v(col, p=128):
                return vec[0:p, col:col + 1]

            # ---- persistent state (feature-major) ----
            xp = spool.tile([HID, BPC * BLK], F32, tag="xp")
            res = spool.tile([HID, BPC * BLK], F32, tag="res")
            xg = spool.tile([HID, BPC * BLK], F32, tag="xg")

            # ---- DRAM table buffers ----
            # combined per-round table rows:
            # [gcn (128) | xh (128) | asrc (h) | adst (h) | pad] = 320 cols
            # Double-buffered across rounds; the AllGather is split in two
            # half-shard collectives (permuted row layout, host remaps the
            # gather indices) so the first half overlaps aggregation.
            TW = 320
            AOFF = 256
            HSH = BPC * BLK // 2          # 1920 rows per half-shard
            agin0a = dram.tile([HSH, TW], F32)
            agin0b = dram.tile([HSH, TW], F32)
            agin1a = dram.tile([HSH, TW], F32)
            agin1b = dram.tile([HSH, TW], F32)
            tbl0 = dram.tile([NPAD, TW], F32)
            tbl1 = dram.tile([NPAD, TW], F32)
            agins = [(agin0a, agin0b), (agin1a, agin1b)]
            tbls = [tbl0, tbl1]

            def blk_sl(b):
                return slice(b * BLK, (b + 1) * BLK)

            # ---- prologue: xp, res, xg from x ----
            for b in range(BPC):
                for (w_sb, bias_col, dst_state) in (
                    (w_in, VC["gcn_in_b"], xp),
                    (w_res, VC["gcn_res_b"], res),
                    (w_gin, VC["gat_in_b"], xg),
                ):
                    p = psC.tile([HID, BLK], F32, tag="tmp")
                    nc.tensor.matmul(out=p[:], lhsT=w_sb[:],
                                     rhs=x_sb[:, blk_sl(b)], start=True, stop=True)
                    nc.scalar.activation(dst_state[:, blk_sl(b)], p[:], AF.Identity,
                                         bias=v(bias_col), scale=1.0)

            # ---- helpers for aggregation ----
            _gq = [0]

            def gather_chunk(tbl_ap, idx_sb, t0, ch, width, elem_step=None):
                g = gpool.tile([128, CHMAX, width], F32,
                               tag=f"g{width}")
                _gq[0] += 1
                nc.gpsimd.dma_gather(
                    g[:, 0:ch, :], tbl_ap, idx_sb[:, t0 * 8:(t0 + ch) * 8],
                    ch * 128, ch * 128, width, single_packet=False,
                    queue_num=_gq[0] % NQ,
                    **({} if elem_step is None else dict(elem_step=elem_step)))
                return g

            def build_S(t0, ch):
                s = wpool.tile([128, CHMAX, 128], F32, tag="S")
                nc.vector.tensor_tensor(
                    out=s[:, 0:ch, :],
                    in0=iota[:].rearrange("p (t k) -> p t k", k=128)[:, 0:ch, :],
                    in1=dl[:, t0:t0 + ch].to_broadcast([128, ch, 128]),
                    op=ALU.is_equal)
                return s

            # ================= merged round: GCN + GAT layer i =================
            def build_table_block(i, b, agin_t):
                h = HEADS[i]
                W2 = HID + 2 * h
                st = stpool.tile([BLK, TW], F32, tag="tb")
                p1 = psB.tile([BLK, HID], F32, tag="tbl")
                nc.tensor.matmul(out=p1[:], lhsT=xp[:, blk_sl(b)], rhs=wg[i][:],
                                 start=True, stop=True)
                nc.scalar.activation(st[:, 0:HID], p1[:], AF.Copy, bias=0.0,
                                     scale=dis[:, b:b + 1])
                p2 = psB.tile([BLK, W2], F32, tag="tbl")
                nc.tensor.matmul(out=p2[:], lhsT=xg[:, blk_sl(b)], rhs=wa[i][:],
                                 start=True, stop=True)
                nc.scalar.activation(st[:, HID:HID + W2], p2[:], AF.Copy,
                                     bias=0.0, scale=1.0)
                hb = b - BPC // 2 if b >= BPC // 2 else b
                nc.sync.dma_start(agin_t[b >= BPC // 2][hb * BLK:(hb + 1) * BLK, :],
                                  st[:])

            def ag_half(agin_t, tbl_t, half):
                nc.gpsimd.collective_compute(
                    "AllGather", ALU.bypass,
                    ins=[agin_t[half].opt()],
                    outs=[tbl_t[half * NC * HSH:(half + 1) * NC * HSH, :]],
                    replica_groups=[list(range(NC))])

            def round_layer(i):
                h = HEADS[i]
                C = HID // h
                tbl = tbls[i % 2]
                agin = agins[i % 2]
                # aggregate per block, both branches off one gathered stream
                for b in range(BPC):
                    # one combined accumulator: [gcn (128) | gat (128) | s (h)]
                    acc = psA.tile([BLK, AOFF + 4], F32, tag="acc")
                    t0 = b * T_blk
                    ti = 0
                    for ci, ch in enumerate(chunks):
                        first = ci == 0
                        last = ci == len(chunks) - 1
                        g = gather_chunk(tbl[:, 0:TW], idxS, t0 + ti, ch, TW)
                        # dst rows are always in this core's own shard (and
                        # within one half): gather adst from the local pre-AG
                        # half buffer (no AG dependency)
                        d = gather_chunk(agin[b >= BPC // 2][:, AOFF:TW],
                                         idxD, t0 + ti, ch, 64, elem_step=TW)
                        s = build_S(t0 + ti, ch)
                        # z = asrc[src] + adst[dst]
                        z = wpool.tile([128, CHMAX, 4], F32, tag="z")
                        nc.vector.tensor_tensor(
                            out=z[:, 0:ch, 0:h], in0=g[:, 0:ch, AOFF:AOFF + h],
                            in1=d[:, 0:ch, h:2 * h], op=ALU.add)
                        # ex = max(exp(z), exp(0.2 z))
                        e1 = wpool.tile([128, CHMAX, 4], F32, tag="e1")
                        e2 = wpool.tile([128, CHMAX, 4], F32, tag="e2")
                        nc.scalar.activation(e1[:, 0:ch, 0:h], z[:, 0:ch, 0:h],
                                             AF.Exp, bias=0.0, scale=1.0)
                        nc.scalar.activation(e2[:, 0:ch, 0:h], z[:, 0:ch, 0:h],
                                             AF.Exp, bias=0.0, scale=0.2)
                        ex = wpool.tile([128, CHMAX, 4], F32, tag="ex")
                        nc.vector.tensor_tensor(out=ex[:, 0:ch, 0:h],
                                                in0=e1[:, 0:ch, 0:h],
                                                in1=e2[:, 0:ch, 0:h], op=ALU.max)
                        # scale xh cols of g in place; overwrite asrc cols by ex:
                        # rhs becomes one contiguous [gcn | ex*xh | ex] slice
                        for hh in range(h):
                            nc.vector.tensor_tensor(
                                out=g[:, 0:ch, HID + hh * C:HID + (hh + 1) * C],
                                in0=g[:, 0:ch, HID + hh * C:HID + (hh + 1) * C],
                                in1=ex[:, 0:ch, hh].to_broadcast([128, ch, C]),
                                op=ALU.mult)
                        nc.scalar.activation(g[:, 0:ch, AOFF:AOFF + h],
                                             ex[:, 0:ch, 0:h], AF.Copy,
                                             bias=0.0, scale=1.0)
                        for t in range(ch):
                            nc.tensor.matmul(
                                out=acc[:, 0:AOFF + h], lhsT=s[:, t, :],
                                rhs=g[:, t, 0:AOFF + h],
                                start=(first and t == 0),
                                stop=(last and t == ch - 1))
                        ti += ch
                    # ---- GCN epilogue ----
                    u = stpool.tile([BLK, HID], F32, tag="gcn_u")
                    nc.vector.tensor_scalar(out=u[:], in0=acc[:, 0:HID],
                                            scalar1=dis[:, b:b + 1], scalar2=None,
                                            op0=ALU.mult)
                    tp = psC.tile([HID, BLK], F32, tag="tmp")
                    nc.tensor.transpose(tp[:], u[:], ident[:])
                    gcol, bcol = VC["gcn_g"][i], VC["gcn_bc"][i]
                    if i == 1:
                        xn = stpool.tile([HID, BLK], F32, tag="gcn_xn")
                        nc.scalar.activation(xn[:], tp[:], AF.Relu,
                                             bias=v(bcol), scale=v(gcol))
                        nc.vector.tensor_tensor(out=xp[:, blk_sl(b)], in0=xn[:],
                                                in1=res[:, blk_sl(b)], op=ALU.add)
                    else:
                        nc.scalar.activation(xp[:, blk_sl(b)], tp[:], AF.Relu,
                                             bias=v(bcol), scale=v(gcol))
                    # ---- GAT epilogue ----
                    gat_epilogue(i, b, h, C, acc[:, HID:AOFF + h])
                    # ---- software pipeline: next round's table for this
                    # block, and fire the first half-AllGather at mid-round
                    if i + 1 < 3:
                        build_table_block(i + 1, b, agins[(i + 1) % 2])
                        if b == BPC // 2 - 1:
                            ag_half(agins[(i + 1) % 2], tbls[(i + 1) % 2], 0)
                if i + 1 < 3:
                    ag_half(agins[(i + 1) % 2], tbls[(i + 1) % 2], 1)

            # ================= GAT epilogue =================
            def gat_epilogue(i, b, h, C, acc):
                gcol = VC["gat_g"][i]
                bcol = VC["gat_bc"][i]
                # out = acc[:, :HID] / (s + 1e-16); elu(bn(out + b))
                rr = stpool.tile([BLK, 4], F32, tag="rr")
                nc.vector.tensor_scalar(out=rr[0:BLK, 0:h],
                                        in0=acc[:, HID:HID + h],
                                        scalar1=1e-16, scalar2=None,
                                        op0=ALU.add)
                nc.vector.reciprocal(rr[0:BLK, 0:h], rr[0:BLK, 0:h])
                o = stpool.tile([BLK, HID], F32, tag="gat_o")
                for hh in range(h):
                    nc.vector.tensor_scalar(
                        out=o[:, hh * C:(hh + 1) * C],
                        in0=acc[:, hh * C:(hh + 1) * C],
                        scalar1=rr[0:BLK, hh:hh + 1], scalar2=None,
                        op0=ALU.mult)
                tp = psC.tile([HID, BLK], F32, tag="tmp")
                nc.tensor.transpose(tp[:], o[:], ident[:])
                # elu(u) = relu(u) + min(exp(u)-1, 0), u = tp*g' + bc
                r1 = stpool.tile([HID, BLK], F32, tag="gat_r1")
                q = stpool.tile([HID, BLK], F32, tag="gat_q")
                nc.scalar.activation(r1[:], tp[:], AF.Relu,
                                     bias=v(bcol), scale=v(gcol))
                nc.scalar.activation(q[:], tp[:], AF.Exp,
                                     bias=v(bcol), scale=v(gcol))
                nc.vector.tensor_scalar(out=q[:], in0=q[:], scalar1=-1.0,
                                        scalar2=0.0, op0=ALU.add, op1=ALU.min)
                nc.vector.tensor_tensor(out=xg[:, blk_sl(b)], in0=r1[:],
                                        in1=q[:], op=ALU.add)

            # ---- run rounds ----
            for b in range(BPC):
                build_table_block(0, b, agins[0])
            ag_half(agins[0], tbls[0], 0)
            ag_half(agins[0], tbls[0], 1)
            for i in range(min(ngcn, 3)):
                round_layer(i)

            # ---- heads + fusion ----
            for b in range(BPC if do_heads else 0):
                # gcn head
                p1 = psC.tile([64, BLK], F32, tag="tmp")
                nc.tensor.matmul(out=p1[:], lhsT=hw["gcn_c1w"][:],
                                 rhs=xp[:, blk_sl(b)], start=True, stop=True)
                a1 = stpool.tile([64, BLK], F32, tag="a1")
                nc.scalar.activation(a1[:], p1[:], AF.Relu,
                                     bias=v(VC["gcn_c1b"], 64), scale=1.0)
                p2 = psC.tile([32, BLK], F32, tag="tmp")
                nc.tensor.matmul(out=p2[:], lhsT=hw["gcn_c2w"][:], rhs=a1[:],
                                 start=True, stop=True)
                a2 = stpool.tile([32, BLK], F32, tag="a2")
                nc.scalar.activation(a2[:], p2[:], AF.Relu,
                                     bias=v(VC["gcn_c2b"], 32), scale=1.0)
                pf = psF.tile([HID, BLK], F32, tag="fuse")
                nc.tensor.matmul(out=pf[:], lhsT=hw["gcn_c3w"][:], rhs=a2[:],
                                 start=True, stop=False, skip_group_check=True)
                # gat head
                p3 = psC.tile([64, BLK], F32, tag="tmp")
                nc.tensor.matmul(out=p3[:], lhsT=hw["gat_c1w"][:],
                                 rhs=xg[:, blk_sl(b)], start=True, stop=True)
                b1 = stpool.tile([64, BLK], F32, tag="a1")
                nc.scalar.activation(b1[:], p3[:], AF.Relu,
                                     bias=v(VC["gat_c1b"], 64), scale=1.0)
                p4 = psC.tile([32, BLK], F32, tag="tmp")
                nc.tensor.matmul(out=p4[:], lhsT=hw["gat_c2w"][:], rhs=b1[:],
                                 start=True, stop=True)
                b2 = stpool.tile([32, BLK], F32, tag="a2")
                nc.scalar.activation(b2[:], p4[:], AF.Relu,
                                     bias=v(VC["gat_c2b"], 32), scale=1.0)
                nc.tensor.matmul(out=pf[:], lhsT=hw["gat_c3w"][:], rhs=b2[:],
                                 start=False, stop=True, skip_group_check=True)
                fs = stpool.tile([HID, BLK], F32, tag="fs")
                nc.scalar.activation(fs[:], pf[:], AF.Identity,
                                     bias=v(VC["fused_b"]), scale=1.0)
                p5 = psC.tile([64, BLK], F32, tag="tmp")
                nc.tensor.matmul(out=p5[:], lhsT=hw["fin_w"][:], rhs=fs[:],
                                 start=True, stop=True)
                f1 = stpool.tile([64, BLK], F32, tag="a1")
                nc.scalar.activation(f1[:], p5[:], AF.Relu,
                                     bias=v(VC["fin_b"], 64), scale=1.0)
                p6 = psC.tile([2, BLK], F32, tag="tmp")
                nc.tensor.matmul(out=p6[:], lhsT=hw["fin2_w"][:], rhs=f1[:],
                                 start=True, stop=True)
                oo = stpool.tile([2, BLK], F32, tag="oo")
                nc.scalar.activation(oo[:], p6[:], AF.Identity,
                                     bias=v(VC["fin2_b"], 2), scale=1.0)
                nc.sync.dma_start(out_fm[:, blk_sl(b)], oo[:])

            if not do_heads:
                # debug path: dump current states so outputs exist
                for b in range(BPC):
                    oo = stpool.tile([2, BLK], F32, tag="oo")
                    nc.vector.tensor_copy(oo[:], xp[0:2, blk_sl(b)])
                    nc.sync.dma_start(out_fm[:, blk_sl(b)], oo[:])

    nc.compile()
    return nc


# ----------------------------------------------------------------------------
# Host orchestration
# ----------------------------------------------------------------------------

def make_inputs_per_core(inputs, src_pad, dst_pad, dstloc, dis, T_blk):
    TILES = BPC * T_blk

    x = np.asarray(inputs["x"], np.float32)
    xpad = np.zeros((NPAD, IN), np.float32)
    xpad[:N] = x
    chunks_max = max(chunks_of(T_blk))

    g = lambda k: np.asarray(inputs[k], np.float32)

    # vecs
    vecs = np.zeros((128, 32), np.float32)

    def setv(col, arr):
        arr = np.asarray(arr, np.float32).ravel()
        vecs[: arr.shape[0], col] = arr

    VC = dict(gcn_in_b=0, gcn_res_b=1, gat_in_b=2,
              gcn_g=[3, 4, 5], gcn_bc=[6, 7, 8],
              gat_g=[9, 10, 11], gat_bc=[12, 13, 14],
              gcn_c1b=15, gcn_c2b=16, fused_b=17,
              gat_c1b=18, gat_c2b=19, fin_b=20, fin2_b=21)
    setv(VC["gcn_in_b"], g("gcn_in_b"))
    setv(VC["gcn_res_b"], g("gcn_res_b"))
    setv(VC["gat_in_b"], g("gat_in_b"))
    for i in range(3):
        gp = bn_fold(g("gcn_bn_g")[i])
        setv(VC["gcn_g"][i], gp)
        setv(VC["gcn_bc"][i], g("gcn_b")[i] * gp + g("gcn_bn_b")[i])
        gp2 = bn_fold(g("gat_bn_g")[i])
        setv(VC["gat_g"][i], gp2)
        setv(VC["gat_bc"][i], g("gat_b")[i] * gp2 + g("gat_bn_b")[i])
    setv(VC["gcn_c1b"], g("gcn_c1b"))
    setv(VC["gcn_c2b"], g("gcn_c2b"))
    setv(VC["fused_b"], 0.6 * g("gcn_c3b") + 0.4 * g("gat_c3b"))
    setv(VC["gat_c1b"], g("gat_c1b"))
    cg = bn_fold(g("gat_cbn_g"))
    setv(VC["gat_c2b"], g("gat_cbn_b") @ g("gat_c2w") + g("gat_c2b"))
    setv(VC["fin_b"], g("fin_b"))
    fg = bn_fold(g("fin_bn_g"))
    setv(VC["fin2_b"], g("fin_bn_b") @ g("fin2_w") + g("fin2_b"))

    # folded head weights
    gcn_c3wf = 0.6 * g("gcn_c3w")
    gat_c1wf = g("feat_imp")[:, None] * g("gat_c1w")
    gat_c2wf = cg[:, None] * g("gat_c2w")
    gat_c3wf = 0.4 * g("gat_c3w")
    fin2_wf = fg[:, None] * g("fin2_w")

    # gat combined weights [W | wsrc | wdst]
    gat_wgs = []
    for i in range(3):
        W = g("gat_w")[i]
        asrc_bd = make_blockdiag(g("gat_asrc")[i], HEADS[i])
        adst_bd = make_blockdiag(g("gat_adst")[i], HEADS[i])
        gat_wgs.append(np.concatenate([W, W @ asrc_bd, W @ adst_bd], axis=1)
                       .astype(np.float32))

    iota = np.tile(np.arange(128, dtype=np.float32)[None, :], (128, chunks_max))

    in_maps = []
    for c in range(NC):
        b0 = c * BPC
        nodes = slice(b0 * BLK, (b0 + BPC) * BLK)
        # src gathers read the split-AllGather table: remap global node id to
        # the permuted row layout (two half-shard AGs concatenated)
        sg = src_pad[b0:b0 + BPC].reshape(-1, 128)      # [TILES, 128] global
        c2 = sg // (BPC * BLK)
        k2 = sg % (BPC * BLK)
        half = BPC * BLK // 2
        src_c = np.where(k2 < half, c2 * half + k2,
                         NC * half + c2 * half + (k2 - half))
        # adst gathers read the core-local half-shard table: indices local
        # to the half buffer (each dst block lies entirely in one half)
        dst_c = (dst_pad[b0:b0 + BPC].reshape(-1, 128) - b0 * BLK) % half
        im = dict(
            x_fm=np.ascontiguousarray(xpad[nodes].T),
            idx_src=wrap_idx_core(src_c),
            idx_dst=wrap_idx_core(dst_c),
            dstloc=np.ascontiguousarray(
                dstloc[b0:b0 + BPC].reshape(-1, 128).T),   # [128, TILES]
            iota_rep=iota,
            dis=np.ascontiguousarray(dis[nodes].reshape(BPC, BLK).T),
            gcn_in_w=g("gcn_in_w"), gcn_res_w=g("gcn_res_w"),
            gat_in_w=g("gat_in_w"), gcn_w=g("gcn_w"),
            gat_wg0=gat_wgs[0], gat_wg1=gat_wgs[1], gat_wg2=gat_wgs[2],
            vecs=vecs,
            gcn_c1w=g("gcn_c1w"), gcn_c2w=g("gcn_c2w"), gcn_c3w=gcn_c3wf,
            gat_c1w=gat_c1wf, gat_c2w=gat_c2wf, gat_c3w=gat_c3wf,
            fin_w=g("fin_w"), fin2_w=fin2_wf,
        )
        in_maps.append(im)
    return in_maps


_CACHE = {}


def _get_program(T_blk):
    if T_blk not in _CACHE:
        _CACHE[T_blk] = build_program(T_blk)
    return _CACHE[T_blk]


def kernel(**inputs):
    edge_index = np.asarray(inputs["edge_index"])
    src_pad, dst_pad, dstloc, dis, T_blk = preprocess_graph(edge_index)
    in_maps = make_inputs_per_core(inputs, src_pad, dst_pad, dstloc, dis, T_blk)
    nc = _get_program(T_blk)
    res = run_bass_kernel_spmd(nc, in_maps, core_ids=list(range(NC)))
    out = np.concatenate([res.results[c]["out_fm"].T for c in range(NC)], axis=0)
    return np.ascontiguousarray(out[:N]).astype(np.float32)



# revision 15
# speedup vs baseline: 2.0780x; 2.0780x over previous
"""Trainium2 Bass kernel for nn_AdvancedHybridGNN (hybrid GCN+GAT, N=30000, E=600000).

v2 design (vs. baseline):
- bf16 data plane: tables, gathers, one-hot S matrices, aggregation matmuls,
  node states. PSUM accumulation stays fp32.
- Table rows 384 bf16 cols (768B): [gcn 0:128 | xh 128:256 | asrc 256:260 |
  adst 260:264 | pad]. One src gather per edge chunk; NO dst gather.
- Per-edge adst computed on the TensorEngine: a second one-hot S2[d, e]
  (built from a host-streamed replicated dstloc row) matmul'd against the
  block's adst vector (bsb, copied from table builds).
- Round-0 table built LOCALLY on every core from the full x (replicated
  input) via host-fused weights W0c -> no AllGather for round 0.
- Rounds 1/2 tables AllGathered in 6 pieces (5 blocks each); edges sorted
  per block by remapped src row so gather chunks only depend on the table
  prefix they actually read (in_ap row slice => fine-grained deps).
- Variable per-block tile counts (no uniform T_blk padding).
"""
import numpy as np
import sys

sys.path.insert(0, "/opt/trn_rl_repo")

import concourse.bacc as bacc
import concourse.bass as bass
import concourse.mybir as mybir
import concourse.tile as tile
from concourse import library_config
from concourse.masks import make_identity
from concourse.bass_utils import run_bass_kernel_spmd

F32 = mybir.dt.float32
BF16 = mybir.dt.bfloat16
I16 = mybir.dt.int16
AF = mybir.ActivationFunctionType
ALU = mybir.AluOpType

N = 30000
NPAD = 30720
E = 600000
IN = 64
HID = 128
EPS = 1e-5
HEADS = (4, 4, 1)
NC = 8
BLK = 128
NBLK = NPAD // BLK        # 240
BPC = NBLK // NC          # 30 blocks per core
NPIECE = 6                # AG pieces per round
BPP = BPC // NPIECE       # 5 blocks per piece
PSH = BPP * BLK           # 640 rows per core per piece
PROWS = NC * PSH          # 5120 rows per piece in the gathered table
TW = 384                  # table row width (bf16) -> 768B rows
UC = 264                  # used columns: 128 gcn + 128 xh + 4 asrc + 4 adst
AOFF = 256                # asrc offset
NQ = 4                    # SWDGE queues
CHSZ = 12                 # tiles per src-gather chunk


def remap_rows(node):
    """Global node id -> piece-major table row."""
    c2 = node // (BPC * BLK)
    k2 = node % (BPC * BLK)
    q = k2 // PSH
    return q * PROWS + c2 * PSH + (k2 % PSH)


# ----------------------------------------------------------------------------
# Host-side graph preprocessing
# ----------------------------------------------------------------------------

def chunks_of(T_b):
    out = []
    t = T_b
    while t > CHSZ:
        out.append(CHSZ)
        t -= CHSZ
    out.append(t)
    return out


def preprocess_graph(edge_index):
    src = np.concatenate([edge_index[0].astype(np.int64),
                          np.arange(N, dtype=np.int64)])
    dst = np.concatenate([edge_index[1].astype(np.int64),
                          np.arange(N, dtype=np.int64)])
    deg = np.bincount(dst, minlength=NPAD)
    dis = np.zeros(NPAD, np.float32)
    m = deg > 0
    dis[m] = (1.0 / np.sqrt(deg[m].astype(np.float64))).astype(np.float32)

    order = np.argsort(dst, kind="stable")
    src_s = src[order]
    dst_s = dst[order]
    rows_s = remap_rows(src_s)
    blk_of = dst_s // BLK
    counts = np.bincount(blk_of, minlength=NBLK)
    starts = np.zeros(NBLK + 1, np.int64)
    np.cumsum(counts, out=starts[1:])

    # per block: sort edges by remapped src row; pad to multiple of 128
    T_b = np.maximum(1, (counts + BLK - 1) // BLK).astype(np.int64)
    blk_srcrow = []   # [T_b*128] remapped rows (pads -> 0)
    blk_dstloc = []   # [T_b*128] float dstloc (pads -> 255)
    for b in range(NBLK):
        c = counts[b]
        s0 = starts[b]
        r = rows_s[s0:s0 + c]
        dl = (dst_s[s0:s0 + c] - b * BLK)
        o = np.argsort(r, kind="stable")
        r = r[o]
        dl = dl[o]
        npad = T_b[b] * BLK - c
        r = np.concatenate([r, np.zeros(npad, np.int64)])
        dl = np.concatenate([dl, np.full(npad, 255, np.int64)])
        blk_srcrow.append(r)
        blk_dstloc.append(dl)
    return blk_srcrow, blk_dstloc, T_b, dis


def wrap_idx(idx_tiles):
    """idx_tiles: [T, 128] int -> int16 gather layout [128, T*8]."""
    T = idx_tiles.shape[0]
    a = idx_tiles.reshape(T, 8, 16).transpose(2, 0, 1).reshape(16, T * 8)
    return np.tile(a, (8, 1)).astype(np.int16)


def bn_fold(g):
    return g / np.sqrt(1.0 + EPS)


def make_blockdiag(a, heads):
    C = HID // heads
    bd = np.zeros((HID, heads), np.float32)
    for h in range(heads):
        bd[h * C:(h + 1) * C, h] = a[h * C:(h + 1) * C]
    return bd


# ----------------------------------------------------------------------------
# Device program
# ----------------------------------------------------------------------------

def build_program(plan):
    """plan: dict with per-core-invariant structure:
       tb:   [BPC] tiles per own block (same for all cores? NO - per core!)
    SPMD: the program must be identical across cores. Block tile counts
    differ per core, so we compile with the per-core MAX profile? No --
    instead the plan carries the GLOBAL per-block profile and each core's
    program uses its own... We compile ONE program; to keep it SPMD we use
    the maximum tiles per block position across cores and pad. See
    make_plan(): tbp[b] = max over cores of T_b[core*BPC+b].
    """
    tbp = plan["tbp"]            # [BPC] tiles for own block b (max over cores)
    deps = plan["deps"]          # [BPC][nchunks] table-prefix pieces (1..6)
    TILES = int(sum(tbp))
    toff = np.zeros(BPC + 1, np.int64)
    np.cumsum(tbp, out=toff[1:])

    nc = bacc.Bacc("TRN2", num_swdge_queues=NQ)

    # ---- inputs ----
    x_aug = nc.dram_tensor("x_aug", [IN + 1, NPAD], BF16, kind="ExternalInput")
    x_own = nc.dram_tensor("x_own", [IN + 1, BPC * BLK], BF16, kind="ExternalInput")
    idx_src = nc.dram_tensor("idx_src", [128, TILES * 8], I16, kind="ExternalInput")
    dstloc = nc.dram_tensor("dstloc", [128, TILES], BF16, kind="ExternalInput")
    dlrow = nc.dram_tensor("dlrow", [128, TILES * BLK], BF16, kind="ExternalInput")
    iota_rep = nc.dram_tensor("iota_rep", [128, CHSZ * 128], BF16, kind="ExternalInput")
    iota_col = nc.dram_tensor("iota_col", [128, 1], BF16, kind="ExternalInput")
    dis_all = nc.dram_tensor("dis_all", [128, NBLK], F32, kind="ExternalInput")
    dis_own = nc.dram_tensor("dis_own", [128, BPC], F32, kind="ExternalInput")

    w0c = nc.dram_tensor("w0c", [IN + 1, UC], BF16, kind="ExternalInput")
    waug_in = nc.dram_tensor("waug_in", [IN + 1, HID], BF16, kind="ExternalInput")
    waug_res = nc.dram_tensor("waug_res", [IN + 1, HID], BF16, kind="ExternalInput")
    waug_gat = nc.dram_tensor("waug_gat", [IN + 1, HID], BF16, kind="ExternalInput")
    wa0_adst = nc.dram_tensor("wa0_adst", [HID, 4], BF16, kind="ExternalInput")
    gcn_w12 = nc.dram_tensor("gcn_w12", [2, HID, HID], BF16, kind="ExternalInput")
    gat_wg1 = nc.dram_tensor("gat_wg1", [HID, HID + 2 * HEADS[1]], BF16,
                             kind="ExternalInput")
    gat_wg2 = nc.dram_tensor("gat_wg2", [HID, HID + 2 * HEADS[2]], BF16,
                             kind="ExternalInput")
    vecs = nc.dram_tensor("vecs", [128, 32], F32, kind="ExternalInput")
    hwts = {}
    for nm, shp in (("gcn_c1w", [HID, 64]), ("gcn_c2w", [64, 32]),
                    ("gcn_c3w", [32, HID]), ("gat_c1w", [HID, 64]),
                    ("gat_c2w", [64, 32]), ("gat_c3w", [32, HID]),
                    ("fin_w", [HID, 64]), ("fin2_w", [64, 2])):
        hwts[nm] = nc.dram_tensor(nm, shp, BF16, kind="ExternalInput")

    out_fm = nc.dram_tensor("out_fm", [2, BPC * BLK], F32, kind="ExternalOutput")

    VC = dict(gcn_g=[3, 4, 5], gcn_bc=[6, 7, 8],
              gat_g=[9, 10, 11], gat_bc=[12, 13, 14],
              gcn_c1b=15, gcn_c2b=16, fused_b=17,
              gat_c1b=18, gat_c2b=19, fin_b=20, fin2_b=21)

    with tile.TileContext(nc) as tc:
        with (
            tc.tile_pool(name="const", bufs=1) as cpool,
            tc.tile_pool(name="state", bufs=1) as spool,
            tc.tile_pool(name="xs", bufs=2) as xpool,
            tc.tile_pool(name="work", bufs=3) as wpool,
            tc.tile_pool(name="s2p", bufs=3) as s2pool,
            tc.tile_pool(name="dlr", bufs=3) as dlpool,
            tc.tile_pool(name="gath", bufs=4) as gpool,
            tc.tile_pool(name="stage", bufs=3) as stpool,
            tc.tile_pool(name="psA", bufs=2, space="PSUM") as psA,   # agg acc
            tc.tile_pool(name="psB", bufs=1, space="PSUM") as psB,   # table mm
            tc.tile_pool(name="psC", bufs=2, space="PSUM") as psC,   # transients
            tc.tile_pool(name="psD", bufs=2, space="PSUM") as psD,   # adst acc
            tc.tile_pool(name="psF", bufs=1, space="PSUM") as psF,   # fusion acc
            tc.tile_pool(name="dram", bufs=1, space="DRAM") as dram,
        ):
            nc.gpsimd.load_library(library_config.mlp)

            # ---- constants into SBUF ----
            _ldn = [0]

            def ld(shape, dt, src):
                _ldn[0] += 1
                t = cpool.tile(shape, dt, tag=f"c{_ldn[0]}")
                nc.sync.dma_start(t[:], src)
                return t

            idxS = ld([128, TILES * 8], I16, idx_src[:])
            dl = ld([128, TILES], BF16, dstloc[:])
            iota = ld([128, CHSZ * 128], BF16, iota_rep[:])
            iotac = ld([128, 1], BF16, iota_col[:])
            disA = ld([128, NBLK], F32, dis_all[:])
            disO = ld([128, BPC], F32, dis_own[:])
            vec = ld([128, 32], F32, vecs[:])
            w0 = ld([IN + 1, UC], BF16, w0c[:])
            w_in = ld([IN + 1, HID], BF16, waug_in[:])
            w_res = ld([IN + 1, HID], BF16, waug_res[:])
            w_gin = ld([IN + 1, HID], BF16, waug_gat[:])
            wa0a = ld([HID, 4], BF16, wa0_adst[:])
            wg = [None,
                  ld([HID, HID], BF16, gcn_w12[0, :, :]),
                  ld([HID, HID], BF16, gcn_w12[1, :, :])]
            wa = [None,
                  ld([HID, HID + 2 * HEADS[1]], BF16, gat_wg1[:]),
                  ld([HID, HID + 2 * HEADS[2]], BF16, gat_wg2[:])]
            hshapes = dict(gcn_c1w=[HID, 64], gcn_c2w=[64, 32],
                           gcn_c3w=[32, HID], gat_c1w=[HID, 64],
                           gat_c2w=[64, 32], gat_c3w=[32, HID],
                           fin_w=[HID, 64], fin2_w=[64, 2])
            hw = {nm: ld(hshapes[nm], BF16, t[:]) for nm, t in hwts.items()}
            x_ow = ld([IN + 1, BPC * BLK], BF16, x_own[:])
            ident = cpool.tile([128, 128], BF16, tag="ident")
            make_identity(nc, ident[:])

            def v(col, p=128):
                return vec[0:p, col:col + 1]

            # ---- persistent state (feature-major, bf16) ----
            xp = spool.tile([HID, BPC * BLK], BF16, tag="xp")
            res = spool.tile([HID, BPC * BLK], BF16, tag="res")
            xg = spool.tile([HID, BPC * BLK], BF16, tag="xg")
            # per-round adst of own nodes: [128, BPC*4], rounds alternate
            bsb_a = spool.tile([128, BPC * 4], BF16, tag="bsb_a")
            bsb_b = spool.tile([128, BPC * 4], BF16, tag="bsb_b")
            bsb = [bsb_a, bsb_b]

            # ---- DRAM buffers ----
            tbl0 = dram.tile([NPAD, TW], BF16)
            tbl1 = dram.tile([NPAD, TW], BF16)
            tbl2 = dram.tile([NPAD, TW], BF16)
            tbl = [tbl0, tbl1, tbl2]
            agin1 = dram.tile([BPC * BLK, TW], BF16)
            agin2 = dram.tile([BPC * BLK, TW], BF16)
            agin = [agin1, agin2]

            def blk_sl(b):
                return slice(b * BLK, (b + 1) * BLK)

            # ================= phase A: local full table-0 build ===========
            # piece-major global block order; x_aug streamed per piece
            for q in range(NPIECE):
                xa = xpool.tile([IN + 1, NC * PSH], BF16, tag="xa")
                nc.sync.dma_start(xa[:], x_aug[:, q * PROWS:(q + 1) * PROWS])
                for j in range(NC * BPP):
                    bg = q * NC * BPP + j
                    p = psB.tile([BLK, UC], F32, tag="tb")
                    nc.tensor.matmul(out=p[:], lhsT=xa[:, blk_sl(j)],
                                     rhs=w0[:], start=True, stop=True)
                    st = stpool.tile([BLK, TW], BF16, tag="tb0s")
                    nc.scalar.activation(st[:, 0:HID], p[:, 0:HID], AF.Copy,
                                         bias=0.0, scale=disA[:, bg:bg + 1])
                    nc.scalar.activation(st[:, HID:UC], p[:, HID:UC], AF.Copy,
                                         bias=0.0, scale=1.0)
                    nc.sync.dma_start(tbl[0][bg * BLK:(bg + 1) * BLK, :],
                                      st[:])

            # ================= prologue: own states + bsb0 =================
            for b in range(BPC):
                for (w_sb, dst_state) in ((w_in, xp), (w_res, res),
                                          (w_gin, xg)):
                    p = psC.tile([HID, BLK], F32, tag="tmp")
                    nc.tensor.matmul(out=p[:], lhsT=w_sb[:],
                                     rhs=x_ow[:, blk_sl(b)], start=True,
                                     stop=True)
                    nc.scalar.activation(dst_state[:, blk_sl(b)], p[:],
                                         AF.Copy, bias=0.0, scale=1.0)
                # bsb0 for round 0: adst of own nodes = xg_blk^T @ wa0_adst
                pb = psC.tile([BLK, 4], F32, tag="tmp")
                nc.tensor.matmul(out=pb[:], lhsT=xg[:, blk_sl(b)], rhs=wa0a[:],
                                 start=True, stop=True)
                nc.scalar.activation(bsb[0][:, b * 4:(b + 1) * 4], pb[:],
                                     AF.Copy, bias=0.0, scale=1.0)

            # ================= helpers =================
            _gq = [0]

            def build_table_block(i, b):
                """Build next-round (i in {1,2}) staging rows for own block b
                into agin[i-1]; also copy adst cols into bsb[i%2]."""
                h = HEADS[i]
                W2 = HID + 2 * h
                st = stpool.tile([BLK, TW], BF16, tag="tb")
                p1 = psB.tile([BLK, HID], F32, tag="tb")
                nc.tensor.matmul(out=p1[:], lhsT=xp[:, blk_sl(b)],
                                 rhs=wg[i][:], start=True, stop=True)
                nc.scalar.activation(st[:, 0:HID], p1[:], AF.Copy, bias=0.0,
                                     scale=disO[:, b:b + 1])
                p2 = psB.tile([BLK, W2], F32, tag="tb")
                nc.tensor.matmul(out=p2[:], lhsT=xg[:, blk_sl(b)],
                                 rhs=wa[i][:], start=True, stop=True)
                nc.scalar.activation(st[:, HID:HID + W2], p2[:], AF.Copy,
                                     bias=0.0, scale=1.0)
                if h < 4:
                    # zero-fill unused asrc/adst cols so layout is uniform
                    nc.vector.memset(st[:, HID + W2:UC], 0.0)
                nc.scalar.activation(bsb[i % 2][:, b * 4:b * 4 + h],
                                     st[:, AOFF + h:AOFF + 2 * h], AF.Copy,
                                     bias=0.0, scale=1.0)
                nc.sync.dma_start(agin[i - 1][blk_sl(b), :], st[:, 0:TW])

            def fire_ag(i, q):
                nc.gpsimd.collective_compute(
                    "AllGather", ALU.bypass,
                    ins=[agin[i - 1][q * PSH:(q + 1) * PSH, :].opt()],
                    outs=[tbl[i][q * PROWS:(q + 1) * PROWS, :]],
                    replica_groups=[list(range(NC))])

            # ================= round =================
            def round_layer(r):
                h = HEADS[r]
                C = HID // h
                for b in range(BPC):
                    T_b = int(tbp[b])
                    chs = chunks_of(T_b)
                    acc = psA.tile([BLK, UC], F32, tag="acc")
                    ti = 0
                    for ci, ch in enumerate(chs):
                        first = ci == 0
                        last = ci == len(chs) - 1
                        t0 = int(toff[b]) + ti
                        dep = int(deps[b][ci])
                        # src gather (bf16 768B rows)
                        g = gpool.tile([128, CHSZ, TW], BF16, tag="g")
                        _gq[0] += 1
                        nc.gpsimd.dma_gather(
                            g[:, 0:ch, :], tbl[r][0:dep * PROWS, :],
                            idxS[:, t0 * 8:(t0 + ch) * 8],
                            ch * 128, ch * 128, TW, single_packet=False,
                            queue_num=_gq[0] % NQ)
                        # S one-hot [e, d]
                        s = wpool.tile([128, CHSZ, 128], BF16, tag="S")
                        nc.vector.tensor_tensor(
                            out=s[:, 0:ch, :],
                            in0=iota[:].rearrange("p (t k) -> p t k",
                                                  k=128)[:, 0:ch, :],
                            in1=dl[:, t0:t0 + ch].to_broadcast([128, ch, 128]),
                            op=ALU.is_equal)
                        # S2 one-hot [d, e] from streamed replicated dstloc
                        dlr = dlpool.tile([128, CHSZ * 128], BF16, tag="dlr")
                        nc.sync.dma_start(dlr[:, 0:ch * 128],
                                          dlrow[:, t0 * 128:(t0 + ch) * 128])
                        s2 = s2pool.tile([128, CHSZ, 128], BF16, tag="S2")
                        nc.vector.tensor_tensor(
                            out=s2[:, 0:ch, :],
                            in0=iotac[:].to_broadcast([128, ch, 128]),
                            in1=dlr[:].rearrange("p (t k) -> p t k",
                                                 k=128)[:, 0:ch, :],
                            op=ALU.is_equal)
                        # per-edge adst via PE: adstP[e, 4t:4t+4]
                        adstP = psD.tile([128, CHSZ * 4], F32, tag="adst")
                        for t in range(ch):
                            nc.tensor.matmul(
                                out=adstP[:, t * 4:t * 4 + h],
                                lhsT=s2[:, t, :],
                                rhs=bsb[r % 2][:, b * 4:b * 4 + h],
                                start=True, stop=True, skip_group_check=True)
                        # z = asrc[src] + adst[dst]; ex = max(exp z, exp .2z)
                        z = wpool.tile([128, CHSZ, 4], BF16, tag="z")
                        nc.vector.tensor_tensor(
                            out=z[:, 0:ch, 0:h],
                            in0=g[:, 0:ch, AOFF:AOFF + h],
                            in1=adstP[:].rearrange("p (t k) -> p t k",
                                                   k=4)[:, 0:ch, 0:h],
                            op=ALU.add)
                        e1 = wpool.tile([128, CHSZ, 4], BF16, tag="e1")
                        e2 = wpool.tile([128, CHSZ, 4], BF16, tag="e2")
                        nc.scalar.activation(e1[:, 0:ch, 0:h], z[:, 0:ch, 0:h],
                                             AF.Exp, bias=0.0, scale=1.0)
                        nc.scalar.activation(e2[:, 0:ch, 0:h], z[:, 0:ch, 0:h],
                                             AF.Exp, bias=0.0, scale=0.2)
                        ex = wpool.tile([128, CHSZ, 4], BF16, tag="ex")
                        nc.vector.tensor_tensor(out=ex[:, 0:ch, 0:h],
                                                in0=e1[:, 0:ch, 0:h],
                                                in1=e2[:, 0:ch, 0:h],
                                                op=ALU.max)
                        # scale xh by ex; write ex into asrc cols
                        for hh in range(h):
                            nc.vector.tensor_tensor(
                                out=g[:, 0:ch, HID + hh * C:HID + (hh + 1) * C],
                                in0=g[:, 0:ch, HID + hh * C:HID + (hh + 1) * C],
                                in1=ex[:, 0:ch, hh].to_broadcast([128, ch, C]),
                                op=ALU.mult)
                        nc.scalar.activation(g[:, 0:ch, AOFF:AOFF + h],
                                             ex[:, 0:ch, 0:h], AF.Copy,
                                             bias=0.0, scale=1.0)
                        # aggregate
                        for t in range(ch):
                            nc.tensor.matmul(
                                out=acc[:, 0:AOFF + h], lhsT=s[:, t, :],
                                rhs=g[:, t, 0:AOFF + h],
                                start=(first and t == 0),
                                stop=(last and t == ch - 1))
                        ti += ch
                    # ---- GCN epilogue ----
                    u = stpool.tile([BLK, HID], BF16, tag="gcn_u")
                    nc.vector.tensor_scalar(out=u[:], in0=acc[:, 0:HID],
                                            scalar1=disO[:, b:b + 1],
                                            scalar2=None, op0=ALU.mult)
                    tp = psC.tile([HID, BLK], BF16, tag="tmp")
                    nc.tensor.transpose(tp[:], u[:], ident[:])
                    gcol, bcol = VC["gcn_g"][r], VC["gcn_bc"][r]
                    if r == 1:
                        xn = stpool.tile([HID, BLK], BF16, tag="gcn_xn")
                        nc.scalar.activation(xn[:], tp[:], AF.Relu,
                                             bias=v(bcol), scale=v(gcol))
                        nc.vector.tensor_tensor(out=xp[:, blk_sl(b)],
                                                in0=xn[:],
                                                in1=res[:, blk_sl(b)],
                                                op=ALU.add)
                    else:
                        nc.scalar.activation(xp[:, blk_sl(b)], tp[:], AF.Relu,
                                             bias=v(bcol), scale=v(gcol))
                    # ---- GAT epilogue ----
                    rr = stpool.tile([BLK, 4], F32, tag="rr")
                    nc.vector.tensor_scalar(out=rr[0:BLK, 0:h],
                                            in0=acc[:, AOFF:AOFF + h],
                                            scalar1=1e-16, scalar2=None,
                                            op0=ALU.add)
                    nc.vector.reciprocal(rr[0:BLK, 0:h], rr[0:BLK, 0:h])
                    o = stpool.tile([BLK, HID], BF16, tag="gat_o")
                    for hh in range(h):
                        nc.vector.tensor_scalar(
                            out=o[:, hh * C:(hh + 1) * C],
                            in0=acc[:, HID + hh * C:HID + (hh + 1) * C],
                            scalar1=rr[0:BLK, hh:hh + 1], scalar2=None,
                            op0=ALU.mult)
                    tp2 = psC.tile([HID, BLK], BF16, tag="tmp")
                    nc.tensor.transpose(tp2[:], o[:], ident[:])
                    gcol, bcol = VC["gat_g"][r], VC["gat_bc"][r]
                    # elu(u) = relu(u) - relu(1 - exp(u)) for u = tp2*g'+bc
                    r1 = stpool.tile([HID, BLK], BF16, tag="gat_r1")
                    qq = stpool.tile([HID, BLK], BF16, tag="gat_q")
                    r2 = stpool.tile([HID, BLK], BF16, tag="gat_r2")
                    nc.scalar.activation(r1[:], tp2[:], AF.Relu,
                                         bias=v(bcol), scale=v(gcol))
                    nc.scalar.activation(qq[:], tp2[:], AF.Exp,
                                         bias=v(bcol), scale=v(gcol))
                    nc.scalar.activation(r2[:], qq[:], AF.Relu,
                                         bias=1.0, scale=-1.0)
                    nc.vector.tensor_tensor(out=xg[:, blk_sl(b)], in0=r1[:],
                                            in1=r2[:], op=ALU.subtract)
                    # ---- next-round table rows + piece AG ----
                    if r < 2:
                        build_table_block(r + 1, b)
                        if b % BPP == BPP - 1:
                            fire_ag(r + 1, b // BPP)

            for r in range(3):
                round_layer(r)

            # ================= heads + fusion =================
            for b in range(BPC):
                p1 = psC.tile([64, BLK], F32, tag="tmp")
                nc.tensor.matmul(out=p1[:], lhsT=hw["gcn_c1w"][:],
                                 rhs=xp[:, blk_sl(b)], start=True, stop=True)
                a1 = stpool.tile([64, BLK], BF16, tag="a1")
                nc.scalar.activation(a1[:], p1[:], AF.Relu,
                                     bias=v(VC["gcn_c1b"], 64), scale=1.0)
                p2 = psC.tile([32, BLK], F32, tag="tmp")
                nc.tensor.matmul(out=p2[:], lhsT=hw["gcn_c2w"][:], rhs=a1[:],
                                 start=True, stop=True)
                a2 = stpool.tile([32, BLK], BF16, tag="a2")
                nc.scalar.activation(a2[:], p2[:], AF.Relu,
                                     bias=v(VC["gcn_c2b"], 32), scale=1.0)
                pf = psF.tile([HID, BLK], F32, tag="fuse")
                nc.tensor.matmul(out=pf[:], lhsT=hw["gcn_c3w"][:], rhs=a2[:],
                                 start=True, stop=False, skip_group_check=True)
                p3 = psC.tile([64, BLK], F32, tag="tmp")
                nc.tensor.matmul(out=p3[:], lhsT=hw["gat_c1w"][:],
                                 rhs=xg[:, blk_sl(b)], start=True, stop=True)
                b1 = stpool.tile([64, BLK], BF16, tag="a1")
                nc.scalar.activation(b1[:], p3[:], AF.Relu,
                                     bias=v(VC["gat_c1b"], 64), scale=1.0)
                p4 = psC.tile([32, BLK], F32, tag="tmp")
                nc.tensor.matmul(out=p4[:], lhsT=hw["gat_c2w"][:], rhs=b1[:],
                                 start=True, stop=True)
                b2 = stpool.tile([32, BLK], BF16, tag="a2")
                nc.scalar.activation(b2[:], p4[:], AF.Relu,
                                     bias=v(VC["gat_c2b"], 32), scale=1.0)
                nc.tensor.matmul(out=pf[:], lhsT=hw["gat_c3w"][:], rhs=b2[:],
                                 start=False, stop=True, skip_group_check=True)
                fs = stpool.tile([HID, BLK], BF16, tag="fs")
                nc.scalar.activation(fs[:], pf[:], AF.Identity,
                                     bias=v(VC["fused_b"]), scale=1.0)
                p5 = psC.tile([64, BLK], F32, tag="tmp")
                nc.tensor.matmul(out=p5[:], lhsT=hw["fin_w"][:], rhs=fs[:],
                                 start=True, stop=True)
                f1 = stpool.tile([64, BLK], BF16, tag="a1")
                nc.scalar.activation(f1[:], p5[:], AF.Relu,
                                     bias=v(VC["fin_b"], 64), scale=1.0)
                p6 = psC.tile([2, BLK], F32, tag="tmp")
                nc.tensor.matmul(out=p6[:], lhsT=hw["fin2_w"][:], rhs=f1[:],
                                 start=True, stop=True)
                oo = stpool.tile([2, BLK], F32, tag="oo")
                nc.scalar.activation(oo[:], p6[:], AF.Identity,
                                     bias=v(VC["fin2_b"], 2), scale=1.0)
                nc.sync.dma_start(out_fm[:, blk_sl(b)], oo[:])

    nc.compile()
    return nc


# ----------------------------------------------------------------------------
# Host orchestration
# ----------------------------------------------------------------------------

def prepare(inputs):
    edge_index = np.asarray(inputs["edge_index"])
    blk_srcrow, blk_dstloc, T_b, dis = preprocess_graph(edge_index)
    tbp = np.max(T_b.reshape(NC, BPC), axis=0)
    TILES = int(tbp.sum())
    toff = np.zeros(BPC + 1, np.int64)
    np.cumsum(tbp, out=toff[1:])

    # pad each block's arrays up to the SPMD profile tbp (extra pad edges)
    rows_pc = np.zeros((NC, TILES * BLK), np.int64)
    dloc_pc = np.full((NC, TILES * BLK), 255, np.int64)
    for c in range(NC):
        for b in range(BPC):
            gb = c * BPC + b
            r = blk_srcrow[gb]
            d = blk_dstloc[gb]
            o0 = int(toff[b]) * BLK
            rows_pc[c, o0:o0 + r.shape[0]] = r
            dloc_pc[c, o0:o0 + d.shape[0]] = d

    # per-chunk deps: max over cores of max row in chunk -> pieces
    deps = []
    for b in range(BPC):
        chs = chunks_of(int(tbp[b]))
        bd = []
        ti = 0
        for ch in chs:
            o0 = (int(toff[b]) + ti) * BLK
            mx = int(rows_pc[:, o0:o0 + ch * BLK].max())
            bd.append(min(NPIECE, mx // PROWS + 1))
            ti += ch
        deps.append(bd)

    plan = dict(tbp=tbp, deps=deps,
                key=(tuple(int(t) for t in tbp),
                     tuple(tuple(d) for d in deps)))

    g = lambda k: np.asarray(inputs[k], np.float32)

    # vecs
    vecs = np.zeros((128, 32), np.float32)

    def setv(col, arr):
        arr = np.asarray(arr, np.float32).ravel()
        vecs[: arr.shape[0], col] = arr

    VC = dict(gcn_g=[3, 4, 5], gcn_bc=[6, 7, 8],
              gat_g=[9, 10, 11], gat_bc=[12, 13, 14],
              gcn_c1b=15, gcn_c2b=16, fused_b=17,
              gat_c1b=18, gat_c2b=19, fin_b=20, fin2_b=21)
    for i in range(3):
        gp = bn_fold(g("gcn_bn_g")[i])
        setv(VC["gcn_g"][i], gp)
        setv(VC["gcn_bc"][i], g("gcn_b")[i] * gp + g("gcn_bn_b")[i])
        gp2 = bn_fold(g("gat_bn_g")[i])
        setv(VC["gat_g"][i], gp2)
        setv(VC["gat_bc"][i], g("gat_b")[i] * gp2 + g("gat_bn_b")[i])
    setv(VC["gcn_c1b"], g("gcn_c1b"))
    setv(VC["gcn_c2b"], g("gcn_c2b"))
    setv(VC["fused_b"], 0.6 * g("gcn_c3b") + 0.4 * g("gat_c3b"))
    setv(VC["gat_c1b"], g("gat_c1b"))
    cg = bn_fold(g("gat_cbn_g"))
    setv(VC["gat_c2b"], g("gat_cbn_b") @ g("gat_c2w") + g("gat_c2b"))
    setv(VC["fin_b"], g("fin_b"))
    fg = bn_fold(g("fin_bn_g"))
    setv(VC["fin2_b"], g("fin_bn_b") @ g("fin2_w") + g("fin2_b"))

    # folded head weights
    gcn_c3wf = 0.6 * g("gcn_c3w")
    gat_c1wf = g("feat_imp")[:, None] * g("gat_c1w")
    gat_c2wf = cg[:, None] * g("gat_c2w")
    gat_c3wf = 0.4 * g("gat_c3w")
    fin2_wf = fg[:, None] * g("fin2_w")

    # gat combined weights [W | Wasrc | Wadst]
    gat_wgs = []
    for i in range(3):
        W = g("gat_w")[i]
        asrc_bd = make_blockdiag(g("gat_asrc")[i], HEADS[i])
        adst_bd = make_blockdiag(g("gat_adst")[i], HEADS[i])
        gat_wgs.append(np.concatenate([W, W @ asrc_bd, W @ adst_bd], axis=1))

    # fused round-0 weights  (ones-row bias trick)
    def aug(Wm, bv):
        return np.concatenate([Wm, bv[None, :]], axis=0)

    w0c = np.zeros((IN + 1, UC), np.float32)
    w0c[:, 0:HID] = aug(g("gcn_in_w") @ g("gcn_w")[0],
                        g("gcn_in_b") @ g("gcn_w")[0])
    w0c[:, HID:HID + 136] = aug(g("gat_in_w") @ gat_wgs[0],
                                g("gat_in_b") @ gat_wgs[0])

    waug_in = aug(g("gcn_in_w"), g("gcn_in_b"))
    waug_res = aug(g("gcn_res_w"), g("gcn_res_b"))
    waug_gat = aug(g("gat_in_w"), g("gat_in_b"))
    wa0_adst = gat_wgs[0][:, 132:136]

    # x in piece-major node order, with ones row
    x = np.zeros((NPAD, IN), np.float32)
    x[:N] = g("x")
    perm = np.zeros(NPAD, np.int64)          # table row -> node
    nodes = np.arange(NPAD, dtype=np.int64)
    perm[remap_rows(nodes)] = nodes
    x_aug = np.concatenate([x[perm].T, np.ones((1, NPAD), np.float32)],
                           axis=0)
    dis_all = np.ascontiguousarray(
        dis[perm].reshape(NBLK, BLK).T).astype(np.float32)

    iota_rep = np.tile(np.arange(128, dtype=np.float32)[None, :],
                       (128, CHSZ))
    iota_col = np.arange(128, dtype=np.float32)[:, None]

    def to_bf(a):
        import jax.numpy as jnp
        return np.asarray(jnp.asarray(np.asarray(a, np.float32)
                                      ).astype(jnp.bfloat16))

    common = dict(
        w0c=to_bf(w0c), waug_in=to_bf(waug_in), waug_res=to_bf(waug_res),
        waug_gat=to_bf(waug_gat), wa0_adst=to_bf(wa0_adst),
        gcn_w12=to_bf(g("gcn_w")[1:3]),
        gat_wg1=to_bf(gat_wgs[1]), gat_wg2=to_bf(gat_wgs[2]),
        vecs=vecs, x_aug=to_bf(x_aug),
        iota_rep=to_bf(iota_rep), iota_col=to_bf(iota_col),
        dis_all=dis_all,
        gcn_c1w=to_bf(g("gcn_c1w")), gcn_c2w=to_bf(g("gcn_c2w")),
        gcn_c3w=to_bf(gcn_c3wf), gat_c1w=to_bf(gat_c1wf),
        gat_c2w=to_bf(gat_c2wf), gat_c3w=to_bf(gat_c3wf),
        fin_w=to_bf(g("fin_w")), fin2_w=to_bf(fin2_wf),
    )

    in_maps = []
    for c in range(NC):
        nsl = slice(c * BPC * BLK, (c + 1) * BPC * BLK)
        x_own = np.concatenate(
            [x[nsl].T, np.ones((1, BPC * BLK), np.float32)], axis=0)
        rows = rows_pc[c].reshape(TILES, BLK)
        dloc = dloc_pc[c].reshape(TILES, BLK)
        im = dict(common)
        im.update(
            x_own=to_bf(x_own),
            idx_src=wrap_idx(rows),
            dstloc=to_bf(np.ascontiguousarray(
                dloc.astype(np.float32).T)),            # [128, TILES]
            dlrow=to_bf(np.tile(
                dloc.astype(np.float32).reshape(1, TILES * BLK),
                (128, 1))),                              # [128, TILES*128]
            dis_own=np.ascontiguousarray(
                dis[nsl].reshape(BPC, BLK).T).astype(np.float32),
        )
        in_maps.append(im)
    return plan, in_maps


_CACHE = {}


def _get_program(plan):
    key = plan["key"]
    if key not in _CACHE:
        _CACHE[key] = build_program(plan)
    return _CACHE[key]


def kernel(**inputs):
    plan, in_maps = prepare(inputs)
    nc = _get_program(plan)
    res = run_bass_kernel_spmd(nc, in_maps, core_ids=list(range(NC)))
    out = np.concatenate([res.results[c]["out_fm"].T for c in range(NC)],
                         axis=0)
    return np.ascontiguousarray(out[:N]).astype(np.float32)
